# revision 8
# baseline (speedup 1.0000x reference)
"""Trainium2 Bass kernel for nn_AttShare: dual-stream 1x1-conv attention.

Full-input contract: kernel(**inputs) takes the complete tensors from
setup_inputs() and returns (out1, out2) exactly like the reference.

Sharding (8 cores): 4 independent (batch, stream) attention units x 2-way
query-row split.  Each core gets the full x=[256,4096] of its unit, HOST-
ROTATED so its 2048 query columns come first; it produces
out = gamma * (V @ softmax(Q K^T)^T)[:, 0:2048] + x[:, 0:2048].
(Attention contracts over all keys, so the key/value column order is
irrelevant; the host scatters the output back to the right columns.)

Key simplification: the reference adds a per-row bias (q . g) to the logits
before a row-softmax.  softmax is shift-invariant per row, so the entire
global-gating branch (pooled means -> MLP -> sigmoid -> bias) cancels and is
not computed.  The k-projection bias also shifts logits uniformly per row
and cancels; the q bias does not and is applied.  The v bias adds
gamma*vb[c] (softmax rows sum to 1); it is folded into the V^T tiles.

Precision: projections and QK logits run in float32r (full fp32 weights,
~19-bit moving operand) -- logit errors get exponentiated, so this path
stays wide.  The PV path (V^T tiles and exp tiles) runs in bfloat16:
weight loads take 1 pass instead of fp32's 2, cutting the ldweights
exposure between back-to-back PV matmuls.  Measured accuracy impact
~1.4e-3 relative (tolerance 2e-2).

On-core dataflow (per core):
  proj:  qq = Wq_dup @ x[:, :2048] (+qb)  [128, 2048] f32r  (q/k duplicated
         kk = Wk_dup @ x  (+kb)           [128, 4096] f32r   on both halves
         vt = gamma*(x^T @ Wv^T + vb)     [128 j, 32, 256] bf16  for packing)
  attn (2 phases of 1024 query columns, j streamed in row-packed pairs,
        software-pipelined one pair ahead):
         S^T tile = kk_j^T @ qq  (K=64, rows 0-63 / 64-127 concurrently)
         E = exp(S^T)  (ScalarE, PSUM -> bf16 SBUF; no max-shift needed:
                        |S|<~60 and the denominator normalizes later)
         ZA += E_A (Pool)   ZB += E_B (Vector)   [split across engines]
         out_psum[c,i] += vt_j^T @ E  (bf16 matmuls, PSUM-resident)
  finalize per 512-col slice (pipelined against the next phase / the last
  PV matmuls): Z colsum+broadcast via all-ones lhsT matmuls (4 terms: ZA,
  ZB and the last pair's exp tiles summed directly by the PE), reciprocal
  (Vector), out = out_psum * recip (Vector) + x (Pool), DMA out (SP/ACT
  queues alternating).
  PSUM budget 8 banks: 4 output accumulators + 2x2-bank S^T tiles.

Head: input DMA issues are split across the SP and Activation hardware
queues (plus Pool's software queue for the scalars) so descriptor
generation (~0.7us each) does not serialize the x stream.
"""

import os
import sys

import numpy as np

for _p in ("/opt/trn_rl_repo", os.path.expanduser("~/.axon_site/_ro/trn_rl_repo")):
    if os.path.isdir(_p) and _p not in sys.path:
        sys.path.insert(0, _p)

import concourse.bass as bass  # noqa: E402
import concourse.bacc as bacc  # noqa: E402
import concourse.mybir as mybir  # noqa: E402
import concourse.tile as tile  # noqa: E402

P = 128
C = 256         # channels
CQ = 64         # q/k channels
N = 4096        # H*W
NI = 2048       # query rows per core
PH = 1024       # query columns processed per phase
B, H, W = 2, 64, 64
F32 = mybir.dt.float32
BF16 = mybir.dt.bfloat16
MM_DT = mybir.dt.float32r


def _f(ap):
    """View a float32r AP as plain fp32 (for non-matmul engine access)."""
    return ap.bitcast(F32)


def _r(ap):
    """View an fp32 AP as float32r (for matmul operands)."""
    return ap.bitcast(MM_DT)


def _emit(tc, aps):
    nc = tc.nc
    import contextlib

    x_d, wq_d, wk_d, wv_d, qb_d, kb_d, vb_d, gamma_d, out_d = aps
    EXP = mybir.ActivationFunctionType.Exp
    IDENT = mybir.ActivationFunctionType.Identity

    with contextlib.ExitStack() as ctx:
        singles = ctx.enter_context(tc.tile_pool(name="singles", bufs=1))
        pp = ctx.enter_context(tc.tile_pool(name="pp", bufs=4, space="PSUM"))
        p_s = ctx.enter_context(tc.tile_pool(name="p_s", bufs=2, space="PSUM"))
        etp = ctx.enter_context(tc.tile_pool(name="etp", bufs=8))
        zp = ctx.enter_context(tc.tile_pool(name="zp", bufs=4))
        outp = ctx.enter_context(tc.tile_pool(name="outp", bufs=4))

        # ---- loads --------------------------------------------------------------
        x_sb = singles.tile([P, 2, N], MM_DT)
        wq_sb = singles.tile([P, 2, P], MM_DT)
        wk_sb = singles.tile([P, 2, P], MM_DT)
        wv_sb = singles.tile([P, 2, C], MM_DT)
        x_r = x_d[:].rearrange("(o p) n -> p o n", p=P)

        gamma_sb = singles.tile([1, 1], F32)
        kb_sb = singles.tile([P, 1], F32)
        qb_sb = singles.tile([P, 1], F32)
        vb_sb = singles.tile([1, C], F32)

        # tiny params on the Pool software queue (complete right after issue,
        # unblocking the broadcast matmuls that give the PE its HAM-warming
        # first work); x chunks split across the two hardware DMA queues so
        # descriptor generation doesn't serialize the stream.
        nc.gpsimd.dma_start(out=gamma_sb, in_=gamma_d[:])
        nc.gpsimd.dma_start(out=vb_sb, in_=vb_d[:])
        nc.gpsimd.dma_start(out=qb_sb, in_=qb_d[:])
        nc.gpsimd.dma_start(out=kb_sb, in_=kb_d[:])

        def ld_x(queue, c):
            queue.dma_start(out=x_sb[:, :, bass.ts(c, N // 8)],
                            in_=x_r[:, :, bass.ts(c, N // 8)])

        # first chunk split by cin-half so the first projection operand lands
        # in ~half the transfer time
        nc.sync.dma_start(out=x_sb[:, 0:1, bass.ts(0, N // 8)],
                          in_=x_r[:, 0:1, bass.ts(0, N // 8)])
        nc.sync.dma_start(out=wq_sb, in_=wq_d[:].rearrange("(o p) m -> p o m", p=P))
        nc.sync.dma_start(out=x_sb[:, 1:2, bass.ts(0, N // 8)],
                          in_=x_r[:, 1:2, bass.ts(0, N // 8)])
        nc.sync.dma_start(out=wk_sb, in_=wk_d[:].rearrange("(o p) m -> p o m", p=P))
        nc.scalar.dma_start(out=wv_sb, in_=wv_d[:].rearrange("(o p) m -> p o m", p=P))
        ld_x(nc.sync, 1)
        ld_x(nc.scalar, 2)
        ld_x(nc.sync, 3)
        ld_x(nc.scalar, 4)
        ld_x(nc.sync, 5)
        ld_x(nc.scalar, 6)
        ld_x(nc.sync, 7)

        ones_jj = singles.tile([P, P], F32)   # all-ones fp32 scratch
        nc.vector.memset(ones_jj, 1.0)
        ones_1 = singles.tile([1, P], F32)    # lhsT for K=1 partition broadcast
        nc.vector.memset(ones_1, 1.0)
        ones_b = singles.tile([P, P], BF16)   # all-ones in bf16 (for et colsum)
        nc.vector.memset(ones_b, 1.0)
        ones_r = singles.tile([P, P], MM_DT)  # all-ones f32r lhsT (for z colsum)
        nc.vector.tensor_copy(ones_r, ones_jj)

        # broadcast gamma and gamma*vb across partitions via K=1 matmuls
        gamma_bc = singles.tile([P, 1], F32)
        pg = pp.tile([P, 1], F32, tag="pp", name="pg")
        nc.tensor.matmul(pg, ones_1, gamma_sb, start=True, stop=True)
        nc.vector.tensor_copy(gamma_bc, pg)
        gvb_bc = singles.tile([P, C], F32)
        pvb = pp.tile([P, C], F32, tag="pp")
        nc.tensor.matmul(pvb, ones_1, vb_sb, start=True, stop=True)
        nc.vector.tensor_scalar_mul(gvb_bc, pvb, gamma_bc)

        # ---- projections --------------------------------------------------------
        qq_sb = singles.tile([P, NI], MM_DT)   # [q; q] duplicated across halves
        kk_sb = singles.tile([P, N], MM_DT)    # [k; k] duplicated across halves
        vt_sb = singles.tile([P, N // P, C], BF16)   # V^T: [j, c], pre-scaled

        def qq_slice(s):
            ps = pp.tile([P, 512], F32, tag="pp", name=f"qq_ps_{s}")
            nc.tensor.matmul(ps, wq_sb[:, 0], x_sb[:, 0, bass.ts(s, 512)],
                             start=True, stop=False)
            nc.tensor.matmul(ps, wq_sb[:, 1], x_sb[:, 1, bass.ts(s, 512)],
                             start=False, stop=True)
            nc.scalar.activation(out=qq_sb[:, bass.ts(s, 512)], in_=ps,
                                 func=IDENT, bias=qb_sb, scale=1.0)

        def kk_slice(s):
            ps = pp.tile([P, 512], F32, tag="pp", name=f"kk_ps_{s}")
            nc.tensor.matmul(ps, wk_sb[:, 0], x_sb[:, 0, bass.ts(s, 512)],
                             start=True, stop=False)
            nc.tensor.matmul(ps, wk_sb[:, 1], x_sb[:, 1, bass.ts(s, 512)],
                             start=False, stop=True)
            nc.scalar.activation(out=kk_sb[:, bass.ts(s, 512)], in_=ps,
                                 func=IDENT, bias=kb_sb, scale=1.0)

        def vt_chunk(j):
            ps = pp.tile([P, C], F32, tag="pp", name=f"vt_ps_{j}")
            nc.tensor.matmul(ps, x_sb[:, 0, bass.ts(j, P)], wv_sb[:, 0],
                             start=True, stop=False)
            nc.tensor.matmul(ps, x_sb[:, 1, bass.ts(j, P)], wv_sb[:, 1],
                             start=False, stop=True)
            nc.vector.scalar_tensor_tensor(
                out=vt_sb[:, j], in0=ps, scalar=gamma_bc, in1=gvb_bc,
                op0=mybir.AluOpType.mult, op1=mybir.AluOpType.add)

        # queries are columns 0..NI-1 of the rotated x; consume x strictly in
        # chunk-arrival order (kk slice s and vt chunks 4s..4s+3 share chunk s)
        qq_slice(0)
        qq_slice(1)
        for s in range(N // 512):
            kk_slice(s)
            for j in range(4 * s, 4 * s + 4):
                vt_chunk(j)
            if s == 1:
                qq_slice(2)
            elif s == 2:
                qq_slice(3)

        # ---- attention ----------------------------------------------------------
        # Row-packed QK: pair (jA, jB) = (2t, 2t+1); jA on PE rows 0-63, jB on
        # rows 64-127 (via the duplicated q/k halves), running concurrently.
        NPAIR = N // P // 2   # 16 pairs per phase
        NPH = NI // PH        # 2 phases

        def issue_pair(ph, t):
            i0 = ph * PH
            ab = []
            ets = []
            for h in range(2):
                ps = p_s.tile([P, PH], F32, tag="s", name=f"ps_{ph}_{t}_{h}")
                ab.append(ps)
            for h, j in ((0, 2 * t), (1, 2 * t + 1)):
                lo = h * CQ
                for si in range(PH // 512):
                    nc.tensor.matmul(
                        ab[h][:, bass.ts(si, 512)],
                        kk_sb[lo:lo + CQ, bass.ts(j, P)],
                        qq_sb[lo:lo + CQ, bass.ds(i0 + si * 512, 512)],
                        start=True, stop=True)
                et = etp.tile([P, PH], BF16, tag="et", name=f"et_{ph}_{t}_{h}")
                nc.scalar.activation(out=et, in_=ab[h], func=EXP, scale=1.0)
                ets.append(et)
            return ets

        state = {}

        def pv_half(po, t, h, et):
            j = 2 * t + h
            for cc in range(C // P):
                for si in range(PH // 512):
                    nc.tensor.matmul(
                        po[cc][si],
                        vt_sb[:, j, bass.ts(cc, P)],
                        et[:, bass.ts(si, 512)],
                        start=(t == 0 and h == 0), stop=(t == NPAIR - 1 and h == 1))

        def finalize(st, etA, etB):
            ph, za, zb, po = st[0], st[1], st[2], st[3]
            i0 = ph * PH
            # Z colsum + partition-broadcast via all-ones lhsT matmuls; the
            # last pair's exp tiles are summed directly by the PE (avoids
            # waiting on the accumulation chains).  Finalize is per-512-col
            # slice so the reciprocal/scale/add/DMA chain pipelines against
            # the PE's remaining work.
            for si in range(PH // 512):
                sl = bass.ts(si, 512)
                pzb = p_s.tile([P, 512], F32, tag="s", name=f"pzb_{ph}_{si}")
                nc.tensor.matmul(pzb, ones_r, za[:, sl],
                                 start=True, stop=False)
                nc.tensor.matmul(pzb, ones_r, zb[:, sl],
                                 start=False, stop=False)
                nc.tensor.matmul(pzb, ones_b, etA[:, sl], start=False, stop=False)
                nc.tensor.matmul(pzb, ones_b, etB[:, sl], start=False, stop=True)
                zbc = zp.tile([P, 512], F32, tag="zbc", name=f"zbc_{ph}_{si}")
                scr = zp.tile([P, 512], F32, tag="scr", name=f"scr_{ph}_{si}")
                nc.vector.reciprocal_approx_accurate(out=zbc, in_=pzb, scratch=scr)
                for cc in range(C // P):
                    sl_i = bass.ds(i0 + si * 512, 512)
                    ob = outp.tile([P, 512], F32, tag="ob", name=f"ob_{ph}_{si}_{cc}")
                    nc.vector.tensor_mul(ob, po[cc][si], zbc)
                    nc.gpsimd.tensor_add(ob, ob, _f(x_sb[:, cc, sl_i]))
                    q = nc.sync if cc == 0 else nc.scalar
                    q.dma_start(
                        out=out_d[:].rearrange("(o p) n -> p o n", p=P)[:, cc, sl_i],
                        in_=ob)

        pend = {(0, 0): issue_pair(0, 0)}
        for ph in range(NPH):
            za = zp.tile([P, PH], MM_DT, tag="za", name=f"za_{ph}")
            zb = zp.tile([P, PH], MM_DT, tag="zb", name=f"zb_{ph}")
            po = [[pp.tile([P, 512], F32, tag="pp", name=f"po_{ph}_{cc}_{si}")
                   for si in range(PH // 512)]
                  for cc in range(C // P)]
            state[ph] = (ph, za, zb, po)
            for t in range(NPAIR):
                etA, etB = pend.pop((ph, t))
                pv_half(po, t, 0, etA)
                nxt = (ph, t + 1) if t + 1 < NPAIR else (
                    (ph + 1, 0) if ph + 1 < NPH else None)
                if nxt is not None:
                    pend[nxt] = issue_pair(*nxt)
                pv_half(po, t, 1, etB)
                if t == NPAIR - 1:
                    state[ph] = state[ph] + (etA, etB)
                elif t == 0:
                    nc.gpsimd.tensor_copy(za, etA)
                    nc.vector.tensor_copy(zb, etB)
                else:
                    nc.gpsimd.tensor_add(za, za, etA)
                    nc.vector.tensor_add(zb, zb, etB)
                if ph > 0 and t == 0:
                    st = state.pop(ph - 1)
                    finalize(st[:4], st[4], st[5])
        st = state.pop(NPH - 1)
        finalize(st[:4], st[4], st[5])


def _build_nc():
    nc = bacc.Bacc(trn_type="TRN2", target_bir_lowering=False, debug=False)
    aps = (
        nc.declare_dram_parameter("x", [C, N], MM_DT, isOutput=False),
        nc.declare_dram_parameter("wqT", [C, P], MM_DT, isOutput=False),
        nc.declare_dram_parameter("wkT", [C, P], MM_DT, isOutput=False),
        nc.declare_dram_parameter("wvT", [C, C], MM_DT, isOutput=False),
        nc.declare_dram_parameter("qb", [P, 1], F32, isOutput=False),
        nc.declare_dram_parameter("kb", [P, 1], F32, isOutput=False),
        nc.declare_dram_parameter("vb", [1, C], F32, isOutput=False),
        nc.declare_dram_parameter("gamma", [1, 1], F32, isOutput=False),
        nc.declare_dram_parameter("out", [C, NI], F32, isOutput=True),
    )
    with tile.TileContext(nc) as tc:
        _emit(tc, aps)
    nc.compile()
    return nc


_NC_CACHE = {}


def get_nc():
    if "nc" not in _NC_CACHE:
        _NC_CACHE["nc"] = _build_nc()
    return _NC_CACHE["nc"]


def make_in_maps(inputs):
    """Build the 8 per-core input maps from the full problem inputs."""
    f = np.float32
    x_streams = [
        np.ascontiguousarray(inputs["input1"].reshape(B, C, N), dtype=f),
        np.ascontiguousarray(inputs["input2"].reshape(B, C, N), dtype=f),
    ]
    wsets = []
    for s in ("1", "2"):
        qw = np.asarray(inputs[f"q{s}_w"], dtype=f)
        kw = np.asarray(inputs[f"k{s}_w"], dtype=f)
        vw = np.asarray(inputs[f"v{s}_w"], dtype=f)
        qb = np.asarray(inputs[f"q{s}_b"], dtype=f)
        kb = np.asarray(inputs[f"k{s}_b"], dtype=f)
        vb = np.asarray(inputs[f"v{s}_b"], dtype=f)
        wsets.append(dict(
            wqT=np.ascontiguousarray(np.concatenate([qw, qw], 0).T),
            wkT=np.ascontiguousarray(np.concatenate([kw, kw], 0).T),
            wvT=np.ascontiguousarray(vw.T),
            qb=np.ascontiguousarray(np.concatenate([qb, qb])[:, None]),
            kb=np.ascontiguousarray(np.concatenate([kb, kb])[:, None]),
            vb=np.ascontiguousarray(vb[None, :]),
        ))
    gamma = np.ascontiguousarray(np.asarray(inputs["gamma"], dtype=f).reshape(1, 1))

    in_maps = []
    for core in range(8):
        u, h = core // 2, core % 2
        b, s = u // 2, u % 2
        xs = x_streams[s][b]
        m = dict(wsets[s])
        # rotate so this core's query slice comes first (attention contracts
        # over all keys, so key order is irrelevant)
        if h == 0:
            m["x"] = xs
        else:
            m["x"] = np.ascontiguousarray(
                np.concatenate([xs[:, NI:], xs[:, :NI]], axis=1))
        m["gamma"] = gamma
        in_maps.append(m)
    return in_maps


def assemble(results, inputs):
    """Stitch the 8 per-core [256, 2048] outputs into (out1, out2)."""
    outs = [np.empty((B, C, N), np.float32) for _ in range(2)]
    for core in range(8):
        u, h = core // 2, core % 2
        b, s = u // 2, u % 2
        outs[s][b][:, h * NI:(h + 1) * NI] = results[core]["out"]
    out1 = outs[0].reshape(B, C, H, W)
    out2 = outs[1].reshape(B, C, H, W)
    return out1, out2


def kernel(**inputs):
    from concourse.bass_utils import run_bass_kernel_spmd

    nc = get_nc()
    in_maps = make_in_maps(inputs)
    res = run_bass_kernel_spmd(nc, in_maps, list(range(8)))
    return assemble(res.results, inputs)


# revision 10
# speedup vs baseline: 1.0123x; 1.0123x over previous
"""Trainium2 Bass kernel for nn_AttShare: dual-stream 1x1-conv attention.

Full-input contract: kernel(**inputs) takes the complete tensors from
setup_inputs() and returns (out1, out2) exactly like the reference.

Sharding (8 cores): 4 independent (batch, stream) attention units x 2-way
query-row split.  Each core gets the full x=[256,4096] of its unit, HOST-
ROTATED so its 2048 query columns come first; it produces
out = gamma * (V @ softmax(Q K^T)^T)[:, 0:2048] + x[:, 0:2048].
(Attention contracts over all keys, so the key/value column order is
irrelevant; the host scatters the output back to the right columns.)

Key simplification: the reference adds a per-row bias (q . g) to the logits
before a row-softmax.  softmax is shift-invariant per row, so the entire
global-gating branch (pooled means -> MLP -> sigmoid -> bias) cancels and is
not computed.  The k-projection bias also shifts logits uniformly per row
and cancels; the q bias does not and is applied.  The v bias adds
gamma*vb[c] (softmax rows sum to 1); it is folded into the V^T tiles.

Precision: projections and QK logits run in float32r (full fp32 weights,
~19-bit moving operand) -- logit errors get exponentiated, so this path
stays wide.  The PV path (V^T tiles and exp tiles) runs in bfloat16:
weight loads take 1 pass instead of fp32's 2, cutting the ldweights
exposure between back-to-back PV matmuls.  Measured accuracy impact
~1.4e-3 relative (tolerance 2e-2).

On-core dataflow (per core):
  proj:  qq = Wq_dup @ x[:, :2048] (+qb)  [128, 2048] f32r  (q/k duplicated
         kk = Wk_dup @ x  (+kb)           [128, 4096] f32r   on both halves
         vt = gamma*(x^T @ Wv^T + vb)     [128 j, 32, 256] bf16  for packing)
  attn (2 phases of 1024 query columns, j streamed in row-packed pairs,
        software-pipelined one pair ahead):
         S^T tile = kk_j^T @ qq  (K=64, rows 0-63 / 64-127 concurrently)
         E = exp(S^T)  (ScalarE, PSUM -> bf16 SBUF; no max-shift needed:
                        |S|<~60 and the denominator normalizes later)
         ZA += E_A (Pool)   ZB += E_B (Vector)   [split across engines]
         out_psum[c,i] += vt_j^T @ E  (bf16 matmuls, PSUM-resident)
  finalize per 512-col slice (pipelined against the next phase / the last
  PV matmuls): Z colsum+broadcast via all-ones lhsT matmuls (4 terms: ZA,
  ZB and the last pair's exp tiles summed directly by the PE), reciprocal
  (Vector), out = out_psum * recip (Vector) + x (Pool), DMA out (SP/ACT
  queues alternating).
  PSUM budget 8 banks: 4 output accumulators + 2x2-bank S^T tiles.

Head: input DMA issues are split across the SP and Activation hardware
queues (plus Pool's software queue for the scalars) so descriptor
generation (~0.7us each) does not serialize the x stream.
"""

import os
import sys

import numpy as np

for _p in ("/opt/trn_rl_repo", os.path.expanduser("~/.axon_site/_ro/trn_rl_repo")):
    if os.path.isdir(_p) and _p not in sys.path:
        sys.path.insert(0, _p)

import concourse.bass as bass  # noqa: E402
import concourse.bacc as bacc  # noqa: E402
import concourse.mybir as mybir  # noqa: E402
import concourse.tile as tile  # noqa: E402

P = 128
C = 256         # channels
CQ = 64         # q/k channels
N = 4096        # H*W
NI = 2048       # query rows per core
PH = 1024       # query columns processed per phase
B, H, W = 2, 64, 64
F32 = mybir.dt.float32
BF16 = mybir.dt.bfloat16
MM_DT = mybir.dt.float32r


def _f(ap):
    """View a float32r AP as plain fp32 (for non-matmul engine access)."""
    return ap.bitcast(F32)


def _r(ap):
    """View an fp32 AP as float32r (for matmul operands)."""
    return ap.bitcast(MM_DT)


def _emit(tc, aps):
    nc = tc.nc
    import contextlib

    x_d, wq_d, wk_d, wv_d, qb_d, kb_d, vb_d, gamma_d, out_d = aps
    EXP = mybir.ActivationFunctionType.Exp
    IDENT = mybir.ActivationFunctionType.Identity

    with contextlib.ExitStack() as ctx:
        singles = ctx.enter_context(tc.tile_pool(name="singles", bufs=1))
        pp = ctx.enter_context(tc.tile_pool(name="pp", bufs=4, space="PSUM"))
        p_s = ctx.enter_context(tc.tile_pool(name="p_s", bufs=2, space="PSUM"))
        etp = ctx.enter_context(tc.tile_pool(name="etp", bufs=8))
        zp = ctx.enter_context(tc.tile_pool(name="zp", bufs=4))
        outp = ctx.enter_context(tc.tile_pool(name="outp", bufs=4))

        # ---- loads --------------------------------------------------------------
        x_sb = singles.tile([P, 2, N], MM_DT)
        wq_sb = singles.tile([P, 2, P], MM_DT)
        wk_sb = singles.tile([P, 2, P], MM_DT)
        wv_sb = singles.tile([P, 2, C], MM_DT)
        x_r = x_d[:].rearrange("(o p) n -> p o n", p=P)

        gamma_sb = singles.tile([1, 1], F32)
        kb_sb = singles.tile([P, 1], F32)
        qb_sb = singles.tile([P, 1], F32)
        vb_sb = singles.tile([1, C], F32)

        # gamma/vb lead the SP queue (tiny transfers, unblock the HAM-warming
        # broadcast matmuls), then the x stream; qb/kb ride the Pool software
        # queue.  ScalarE issues no DMAs -- it is exp-bound.
        nc.sync.dma_start(out=gamma_sb, in_=gamma_d[:])
        nc.sync.dma_start(out=vb_sb, in_=vb_d[:])
        nc.gpsimd.dma_start(out=qb_sb, in_=qb_d[:])
        nc.gpsimd.dma_start(out=kb_sb, in_=kb_d[:])

        def ld_x(queue, c):
            queue.dma_start(out=x_sb[:, :, bass.ts(c, N // 8)],
                            in_=x_r[:, :, bass.ts(c, N // 8)])

        # first chunk split by cin-half so the first projection operand lands
        # in ~half the transfer time
        nc.sync.dma_start(out=x_sb[:, 0:1, bass.ts(0, N // 8)],
                          in_=x_r[:, 0:1, bass.ts(0, N // 8)])
        nc.sync.dma_start(out=wq_sb, in_=wq_d[:].rearrange("(o p) m -> p o m", p=P))
        nc.sync.dma_start(out=x_sb[:, 1:2, bass.ts(0, N // 8)],
                          in_=x_r[:, 1:2, bass.ts(0, N // 8)])
        nc.sync.dma_start(out=wk_sb, in_=wk_d[:].rearrange("(o p) m -> p o m", p=P))
        nc.sync.dma_start(out=wv_sb, in_=wv_d[:].rearrange("(o p) m -> p o m", p=P))
        for c in range(1, 8):
            ld_x(nc.sync, c)

        ones_1 = singles.tile([1, P], F32)    # lhsT for K=1 partition broadcast
        nc.vector.memset(ones_1, 1.0)
        ones_b = singles.tile([P, P], BF16)   # all-ones bf16 lhsT: Z colsum
        nc.vector.memset(ones_b, 1.0)

        # broadcast gamma and gamma*vb across partitions via K=1 matmuls
        gamma_bc = singles.tile([P, 1], F32)
        pg = pp.tile([P, 1], F32, tag="pp", name="pg")
        nc.tensor.matmul(pg, ones_1, gamma_sb, start=True, stop=True)
        nc.vector.tensor_copy(gamma_bc, pg)
        gvb_bc = singles.tile([P, C], F32)
        pvb = pp.tile([P, C], F32, tag="pp")
        nc.tensor.matmul(pvb, ones_1, vb_sb, start=True, stop=True)
        nc.vector.tensor_scalar_mul(gvb_bc, pvb, gamma_bc)

        # ---- projections --------------------------------------------------------
        qq_sb = singles.tile([P, NI], MM_DT)   # [q; q] duplicated across halves
        kk_sb = singles.tile([P, N], MM_DT)    # [k; k] duplicated across halves
        vt_sb = singles.tile([P, N // P, C], BF16)   # V^T: [j, c], pre-scaled

        def qq_slice(s):
            ps = pp.tile([P, 512], F32, tag="pp", name=f"qq_ps_{s}")
            nc.tensor.matmul(ps, wq_sb[:, 0], x_sb[:, 0, bass.ts(s, 512)],
                             start=True, stop=False)
            nc.tensor.matmul(ps, wq_sb[:, 1], x_sb[:, 1, bass.ts(s, 512)],
                             start=False, stop=True)
            nc.scalar.activation(out=qq_sb[:, bass.ts(s, 512)], in_=ps,
                                 func=IDENT, bias=qb_sb, scale=1.0)

        def kk_slice(s):
            ps = pp.tile([P, 512], F32, tag="pp", name=f"kk_ps_{s}")
            nc.tensor.matmul(ps, wk_sb[:, 0], x_sb[:, 0, bass.ts(s, 512)],
                             start=True, stop=False)
            nc.tensor.matmul(ps, wk_sb[:, 1], x_sb[:, 1, bass.ts(s, 512)],
                             start=False, stop=True)
            nc.scalar.activation(out=kk_sb[:, bass.ts(s, 512)], in_=ps,
                                 func=IDENT, bias=kb_sb, scale=1.0)

        def vt_chunk(j):
            ps = pp.tile([P, C], F32, tag="pp", name=f"vt_ps_{j}")
            nc.tensor.matmul(ps, x_sb[:, 0, bass.ts(j, P)], wv_sb[:, 0],
                             start=True, stop=False)
            nc.tensor.matmul(ps, x_sb[:, 1, bass.ts(j, P)], wv_sb[:, 1],
                             start=False, stop=True)
            nc.vector.scalar_tensor_tensor(
                out=vt_sb[:, j], in0=ps, scalar=gamma_bc, in1=gvb_bc,
                op0=mybir.AluOpType.mult, op1=mybir.AluOpType.add)

        # queries are columns 0..NI-1 of the rotated x; consume x strictly in
        # chunk-arrival order (kk slice s and vt chunks 4s..4s+3 share chunk s)
        qq_slice(0)
        qq_slice(1)
        for s in range(N // 512):
            kk_slice(s)
            for j in range(4 * s, 4 * s + 4):
                vt_chunk(j)
            if s == 1:
                qq_slice(2)
            elif s == 2:
                qq_slice(3)

        # ---- attention ----------------------------------------------------------
        # Row-packed QK: pair (jA, jB) = (2t, 2t+1); jA on PE rows 0-63, jB on
        # rows 64-127 (via the duplicated q/k halves), running concurrently.
        NPAIR = N // P // 2   # 16 pairs per phase
        NPH = NI // PH        # 2 phases

        def issue_pair(ph, t):
            i0 = ph * PH
            ab = []
            ets = []
            for h in range(2):
                ps = p_s.tile([P, PH], F32, tag="s", name=f"ps_{ph}_{t}_{h}")
                ab.append(ps)
                ets.append(etp.tile([P, PH], BF16, tag="et", name=f"et_{ph}_{t}_{h}"))
            for h, j in ((0, 2 * t), (1, 2 * t + 1)):
                lo = h * CQ
                for si in range(PH // 512):
                    nc.tensor.matmul(
                        ab[h][:, bass.ts(si, 512)],
                        kk_sb[lo:lo + CQ, bass.ts(j, P)],
                        qq_sb[lo:lo + CQ, bass.ds(i0 + si * 512, 512)],
                        start=True, stop=True)
            # exp split per 512-col slice, in PV consumption order, so each PV
            # group's operand is ready as early as possible
            for h in range(2):
                for si in range(PH // 512):
                    sl = bass.ts(si, 512)
                    nc.scalar.activation(out=ets[h][:, sl], in_=ab[h][:, sl],
                                         func=EXP, scale=1.0)
            return ets

        def pv_half(po, t, h, et):
            j = 2 * t + h
            for si in range(PH // 512):
                for cc in range(C // P):
                    nc.tensor.matmul(
                        po[cc][si],
                        vt_sb[:, j, bass.ts(cc, P)],
                        et[:, bass.ts(si, 512)],
                        start=(t == 0 and h == 0), stop=(t == NPAIR - 1 and h == 1))

        def finalize(ph, za, zb, po, etA, etB):
            i0 = ph * PH
            # Z colsum + partition-broadcast via all-ones bf16 lhsT matmuls;
            # the last pair's exp tiles are summed directly by the PE (avoids
            # waiting on the accumulation chains).  Per-512-col slices so the
            # reciprocal/scale/add/DMA chain pipelines against the PE's next
            # phase (or, for the last phase, its remaining slices).
            for si in range(PH // 512):
                sl = bass.ts(si, 512)
                pzb = p_s.tile([P, 512], F32, tag="s", name=f"pzb_{ph}_{si}")
                nc.tensor.matmul(pzb, ones_b, za[:, sl], start=True, stop=False)
                nc.tensor.matmul(pzb, ones_b, zb[:, sl], start=False, stop=False)
                nc.tensor.matmul(pzb, ones_b, etA[:, sl], start=False, stop=False)
                nc.tensor.matmul(pzb, ones_b, etB[:, sl], start=False, stop=True)
                zbc = zp.tile([P, 512], F32, tag="zbc", name=f"zbc_{ph}_{si}")
                scr = zp.tile([P, 512], F32, tag="scr", name=f"scr_{ph}_{si}")
                nc.vector.reciprocal_approx_accurate(out=zbc, in_=pzb, scratch=scr)
                for cc in range(C // P):
                    sl_i = bass.ds(i0 + si * 512, 512)
                    ob = outp.tile([P, 512], F32, tag="ob", name=f"ob_{ph}_{si}_{cc}")
                    nc.vector.tensor_mul(ob, po[cc][si], zbc)
                    nc.gpsimd.tensor_add(ob, ob, _f(x_sb[:, cc, sl_i]))
                    nc.sync.dma_start(
                        out=out_d[:].rearrange("(o p) n -> p o n", p=P)[:, cc, sl_i],
                        in_=ob)

        pend = {(0, 0): issue_pair(0, 0)}
        for ph in range(NPH):
            za = zp.tile([P, PH], BF16, tag="za", name=f"za_{ph}")
            zb = zp.tile([P, PH], BF16, tag="zb", name=f"zb_{ph}")
            po = [[pp.tile([P, 512], F32, tag="pp", name=f"po_{ph}_{cc}_{si}")
                   for si in range(PH // 512)]
                  for cc in range(C // P)]
            for t in range(NPAIR):
                etA, etB = pend.pop((ph, t))
                pv_half(po, t, 0, etA)
                nxt = (ph, t + 1) if t + 1 < NPAIR else (
                    (ph + 1, 0) if ph + 1 < NPH else None)
                if nxt is not None:
                    pend[nxt] = issue_pair(*nxt)
                pv_half(po, t, 1, etB)
                if t == NPAIR - 1:
                    # finalize immediately: the pzb matmuls slot in right
                    # after the last PV, and the Vector chain frees the po
                    # PSUM banks before the next phase's first PV needs them
                    finalize(ph, za, zb, po, etA, etB)
                elif t == 0:
                    nc.vector.tensor_copy(za, etA)
                    nc.vector.tensor_copy(zb, etB)
                else:
                    nc.vector.tensor_add(za, za, etA)
                    nc.vector.tensor_add(zb, zb, etB)


def _build_nc():
    nc = bacc.Bacc(trn_type="TRN2", target_bir_lowering=False, debug=False)
    aps = (
        nc.declare_dram_parameter("x", [C, N], MM_DT, isOutput=False),
        nc.declare_dram_parameter("wqT", [C, P], MM_DT, isOutput=False),
        nc.declare_dram_parameter("wkT", [C, P], MM_DT, isOutput=False),
        nc.declare_dram_parameter("wvT", [C, C], MM_DT, isOutput=False),
        nc.declare_dram_parameter("qb", [P, 1], F32, isOutput=False),
        nc.declare_dram_parameter("kb", [P, 1], F32, isOutput=False),
        nc.declare_dram_parameter("vb", [1, C], F32, isOutput=False),
        nc.declare_dram_parameter("gamma", [1, 1], F32, isOutput=False),
        nc.declare_dram_parameter("out", [C, NI], F32, isOutput=True),
    )
    with tile.TileContext(nc) as tc:
        _emit(tc, aps)
    nc.compile()
    return nc


_NC_CACHE = {}


def get_nc():
    if "nc" not in _NC_CACHE:
        _NC_CACHE["nc"] = _build_nc()
    return _NC_CACHE["nc"]


def make_in_maps(inputs):
    """Build the 8 per-core input maps from the full problem inputs."""
    f = np.float32
    x_streams = [
        np.ascontiguousarray(inputs["input1"].reshape(B, C, N), dtype=f),
        np.ascontiguousarray(inputs["input2"].reshape(B, C, N), dtype=f),
    ]
    wsets = []
    for s in ("1", "2"):
        qw = np.asarray(inputs[f"q{s}_w"], dtype=f)
        kw = np.asarray(inputs[f"k{s}_w"], dtype=f)
        vw = np.asarray(inputs[f"v{s}_w"], dtype=f)
        qb = np.asarray(inputs[f"q{s}_b"], dtype=f)
        kb = np.asarray(inputs[f"k{s}_b"], dtype=f)
        vb = np.asarray(inputs[f"v{s}_b"], dtype=f)
        wsets.append(dict(
            wqT=np.ascontiguousarray(np.concatenate([qw, qw], 0).T),
            wkT=np.ascontiguousarray(np.concatenate([kw, kw], 0).T),
            wvT=np.ascontiguousarray(vw.T),
            qb=np.ascontiguousarray(np.concatenate([qb, qb])[:, None]),
            kb=np.ascontiguousarray(np.concatenate([kb, kb])[:, None]),
            vb=np.ascontiguousarray(vb[None, :]),
        ))
    gamma = np.ascontiguousarray(np.asarray(inputs["gamma"], dtype=f).reshape(1, 1))

    in_maps = []
    for core in range(8):
        u, h = core // 2, core % 2
        b, s = u // 2, u % 2
        xs = x_streams[s][b]
        m = dict(wsets[s])
        # rotate so this core's query slice comes first (attention contracts
        # over all keys, so key order is irrelevant)
        if h == 0:
            m["x"] = xs
        else:
            m["x"] = np.ascontiguousarray(
                np.concatenate([xs[:, NI:], xs[:, :NI]], axis=1))
        m["gamma"] = gamma
        in_maps.append(m)
    return in_maps


def assemble(results, inputs):
    """Stitch the 8 per-core [256, 2048] outputs into (out1, out2)."""
    outs = [np.empty((B, C, N), np.float32) for _ in range(2)]
    for core in range(8):
        u, h = core // 2, core % 2
        b, s = u // 2, u % 2
        outs[s][b][:, h * NI:(h + 1) * NI] = results[core]["out"]
    out1 = outs[0].reshape(B, C, H, W)
    out2 = outs[1].reshape(B, C, H, W)
    return out1, out2


def kernel(**inputs):
    from concourse.bass_utils import run_bass_kernel_spmd

    nc = get_nc()
    in_maps = make_in_maps(inputs)
    res = run_bass_kernel_spmd(nc, in_maps, list(range(8)))
    return assemble(res.results, inputs)


# revision 13
# speedup vs baseline: 1.0899x; 1.0766x over previous
"""Trainium2 Bass kernel for nn_AttShare: dual-stream 1x1-conv attention.

Full-input contract: kernel(**inputs) takes the complete tensors from
setup_inputs() and returns (out1, out2) exactly like the reference.

Sharding (8 cores): 4 independent (batch, stream) attention units x 2-way
query-row split.  Each core gets the full x=[256,4096] of its unit, HOST-
ROTATED so its 2048 query columns come first; it produces
out = gamma * (V @ softmax(Q K^T)^T)[:, 0:2048] + x[:, 0:2048].
(Attention contracts over all keys, so the key/value column order is
irrelevant; the host scatters the output back to the right columns.)

Key simplification: the reference adds a per-row bias (q . g) to the logits
before a row-softmax.  softmax is shift-invariant per row, so the entire
global-gating branch (pooled means -> MLP -> sigmoid -> bias) cancels and is
not computed.  The k-projection bias also shifts logits uniformly per row
and cancels; the q bias does not and is applied.  The v bias adds
gamma*vb[c] (softmax rows sum to 1); it is folded into the V^T tiles.

Precision: projections and QK logits run in float32r (full fp32 weights,
~19-bit moving operand) -- logit errors get exponentiated, so this path
stays wide.  The PV path (V^T tiles and exp tiles) runs in bfloat16:
weight loads take 1 pass instead of fp32's 2, cutting the ldweights
exposure between back-to-back PV matmuls.  Measured accuracy impact
~1.4e-3 relative (tolerance 2e-2).

On-core dataflow (per core):
  proj:  qq = Wq_dup @ x[:, :2048] (+qb)  [128, 2048] f32r  (q/k duplicated
         kk = Wk_dup @ x  (+kb)           [128, 4096] f32r   on both halves
         vt = gamma*(x^T @ Wv^T + vb)     [128 j, 32, 256] bf16  for packing)
  attn (2 phases of 1024 query columns, j streamed in row-packed pairs,
        software-pipelined one pair ahead):
         S^T tile = kk_j^T @ qq  (K=64, rows 0-63 / 64-127 concurrently)
         E = exp(S^T)  (ScalarE, PSUM -> bf16 SBUF; no max-shift needed:
                        |S|<~60 and the denominator normalizes later)
         ZA += E_A (Pool)   ZB += E_B (Vector)   [split across engines]
         out_psum[c,i] += vt_j^T @ E  (bf16 matmuls, PSUM-resident)
  finalize per 512-col slice (pipelined against the next phase / the last
  PV matmuls): Z colsum+broadcast via all-ones lhsT matmuls (4 terms: ZA,
  ZB and the last pair's exp tiles summed directly by the PE), reciprocal
  (Vector), out = out_psum * recip (Vector) + x (Pool), DMA out (SP/ACT
  queues alternating).
  PSUM budget 8 banks: 4 output accumulators + 2x2-bank S^T tiles.

Head: input DMA issues are split across the SP and Activation hardware
queues (plus Pool's software queue for the scalars) so descriptor
generation (~0.7us each) does not serialize the x stream.
"""

import os
import sys

import numpy as np

for _p in ("/opt/trn_rl_repo", os.path.expanduser("~/.axon_site/_ro/trn_rl_repo")):
    if os.path.isdir(_p) and _p not in sys.path:
        sys.path.insert(0, _p)

import concourse.bass as bass  # noqa: E402
import concourse.bacc as bacc  # noqa: E402
import concourse.mybir as mybir  # noqa: E402
import concourse.tile as tile  # noqa: E402

P = 128
C = 256         # channels
CQ = 64         # q/k channels
N = 4096        # H*W
NI = 2048       # query rows per core
PH = 512        # query columns processed per phase
B, H, W = 2, 64, 64
F32 = mybir.dt.float32
BF16 = mybir.dt.bfloat16
MM_DT = mybir.dt.float32r


def _f(ap):
    """View a float32r AP as plain fp32 (for non-matmul engine access)."""
    return ap.bitcast(F32)


def _r(ap):
    """View an fp32 AP as float32r (for matmul operands)."""
    return ap.bitcast(MM_DT)


def _emit(tc, aps):
    nc = tc.nc
    import contextlib

    x_d, wq_d, wk_d, wv_d, qb_d, kb_d, vb_d, gamma_d, out_d = aps
    EXP = mybir.ActivationFunctionType.Exp
    IDENT = mybir.ActivationFunctionType.Identity

    with contextlib.ExitStack() as ctx:
        singles = ctx.enter_context(tc.tile_pool(name="singles", bufs=1))
        pp = ctx.enter_context(tc.tile_pool(name="pp", bufs=4, space="PSUM"))
        p_s = ctx.enter_context(tc.tile_pool(name="p_s", bufs=2, space="PSUM"))
        etp = ctx.enter_context(tc.tile_pool(name="etp", bufs=8))
        zp = ctx.enter_context(tc.tile_pool(name="zp", bufs=4))
        outp = ctx.enter_context(tc.tile_pool(name="outp", bufs=4))

        # ---- loads --------------------------------------------------------------
        x_sb = singles.tile([P, 2, N], MM_DT)
        wq_sb = singles.tile([P, 2, P], MM_DT)
        wk_sb = singles.tile([P, 2, P], MM_DT)
        wv_sb = singles.tile([P, 2, C], MM_DT)
        x_r = x_d[:].rearrange("(o p) n -> p o n", p=P)

        gamma_sb = singles.tile([1, 1], F32)
        kb_sb = singles.tile([P, 1], F32)
        qb_sb = singles.tile([P, 1], F32)
        vb_sb = singles.tile([1, C], F32)

        # gamma/vb lead the SP queue (tiny transfers, unblock the HAM-warming
        # broadcast matmuls), then the x stream; qb/kb ride the Pool software
        # queue.  ScalarE issues no DMAs -- it is exp-bound.
        nc.sync.dma_start(out=gamma_sb, in_=gamma_d[:])
        nc.sync.dma_start(out=vb_sb, in_=vb_d[:])
        nc.gpsimd.dma_start(out=qb_sb, in_=qb_d[:])
        nc.gpsimd.dma_start(out=kb_sb, in_=kb_d[:])

        def ld_x(queue, c):
            queue.dma_start(out=x_sb[:, :, bass.ts(c, N // 8)],
                            in_=x_r[:, :, bass.ts(c, N // 8)])

        # first chunk split by cin-half so the first projection operand lands
        # in ~half the transfer time
        nc.sync.dma_start(out=x_sb[:, 0:1, bass.ts(0, N // 8)],
                          in_=x_r[:, 0:1, bass.ts(0, N // 8)])
        nc.sync.dma_start(out=wq_sb, in_=wq_d[:].rearrange("(o p) m -> p o m", p=P))
        nc.sync.dma_start(out=x_sb[:, 1:2, bass.ts(0, N // 8)],
                          in_=x_r[:, 1:2, bass.ts(0, N // 8)])
        nc.sync.dma_start(out=wk_sb, in_=wk_d[:].rearrange("(o p) m -> p o m", p=P))
        nc.sync.dma_start(out=wv_sb, in_=wv_d[:].rearrange("(o p) m -> p o m", p=P))
        # alternate the two hardware DMA rings so the x stream is not limited
        # by a single ring's bandwidth
        for c in range(1, 8):
            ld_x(nc.sync if c % 2 == 0 else nc.scalar, c)

        ones_1 = singles.tile([1, P], F32)    # lhsT for K=1 partition broadcast
        nc.vector.memset(ones_1, 1.0)
        ones_b = singles.tile([P, P], BF16)   # all-ones bf16 lhsT: Z colsum
        nc.vector.memset(ones_b, 1.0)

        # broadcast gamma and gamma*vb across partitions via K=1 matmuls
        gamma_bc = singles.tile([P, 1], F32)
        pg = pp.tile([P, 1], F32, tag="pp", name="pg")
        nc.tensor.matmul(pg, ones_1, gamma_sb, start=True, stop=True)
        nc.vector.tensor_copy(gamma_bc, pg)
        gvb_bc = singles.tile([P, C], F32)
        pvb = pp.tile([P, C], F32, tag="pp")
        nc.tensor.matmul(pvb, ones_1, vb_sb, start=True, stop=True)
        nc.vector.tensor_scalar_mul(gvb_bc, pvb, gamma_bc)

        # ---- projections --------------------------------------------------------
        qq_sb = singles.tile([P, NI], MM_DT)   # [q; q] duplicated across halves
        kk_sb = singles.tile([P, N], MM_DT)    # [k; k] duplicated across halves
        vt_sb = singles.tile([P, N // P, C], BF16)   # V^T: [j, c], pre-scaled

        def qq_slice(s):
            ps = pp.tile([P, 512], F32, tag="pp", name=f"qq_ps_{s}")
            nc.tensor.matmul(ps, wq_sb[:, 0], x_sb[:, 0, bass.ts(s, 512)],
                             start=True, stop=False)
            nc.tensor.matmul(ps, wq_sb[:, 1], x_sb[:, 1, bass.ts(s, 512)],
                             start=False, stop=True)
            nc.scalar.activation(out=qq_sb[:, bass.ts(s, 512)], in_=ps,
                                 func=IDENT, bias=qb_sb, scale=1.0)

        def kk_slice(s):
            ps = pp.tile([P, 512], F32, tag="pp", name=f"kk_ps_{s}")
            nc.tensor.matmul(ps, wk_sb[:, 0], x_sb[:, 0, bass.ts(s, 512)],
                             start=True, stop=False)
            nc.tensor.matmul(ps, wk_sb[:, 1], x_sb[:, 1, bass.ts(s, 512)],
                             start=False, stop=True)
            nc.scalar.activation(out=kk_sb[:, bass.ts(s, 512)], in_=ps,
                                 func=IDENT, bias=kb_sb, scale=1.0)

        def vt_chunk(j):
            ps = pp.tile([P, C], F32, tag="pp", name=f"vt_ps_{j}")
            nc.tensor.matmul(ps, x_sb[:, 0, bass.ts(j, P)], wv_sb[:, 0],
                             start=True, stop=False)
            nc.tensor.matmul(ps, x_sb[:, 1, bass.ts(j, P)], wv_sb[:, 1],
                             start=False, stop=True)
            nc.vector.scalar_tensor_tensor(
                out=vt_sb[:, j], in0=ps, scalar=gamma_bc, in1=gvb_bc,
                op0=mybir.AluOpType.mult, op1=mybir.AluOpType.add)

        # queries are columns 0..NI-1 of the rotated x; consume x strictly in
        # chunk-arrival order (kk slice s and vt chunks 4s..4s+3 share chunk s)
        qq_slice(0)
        qq_slice(1)
        for s in range(N // 512):
            kk_slice(s)
            for j in range(4 * s, 4 * s + 4):
                vt_chunk(j)
            if s == 1:
                qq_slice(2)
            elif s == 2:
                qq_slice(3)

        # ---- attention ----------------------------------------------------------
        # Row-packed QK: pair (jA, jB) = (2t, 2t+1); jA on PE rows 0-63, jB on
        # rows 64-127 (via the duplicated q/k halves), running concurrently.
        NPAIR = N // P // 2   # 16 pairs per phase
        NPH = NI // PH        # 2 phases

        def issue_pair(ph, t):
            # One PSUM tile holds both halves' S^T slices ([P, 2, 512]); the
            # two K=64 QK matmuls row-pack (rows 0-63 / 64-127) and a SINGLE
            # [128, 1024] exp covers both halves (amortizes ScalarE's fixed
            # per-instruction overhead -- ScalarE is the near-critical engine).
            i0 = ph * PH
            ps = p_s.tile([P, 2, PH], F32, tag="s", name=f"ps_{ph}_{t}")
            for h, j in ((0, 2 * t), (1, 2 * t + 1)):
                lo = h * CQ
                nc.tensor.matmul(
                    ps[:, h],
                    kk_sb[lo:lo + CQ, bass.ts(j, P)],
                    qq_sb[lo:lo + CQ, bass.ds(i0, PH)],
                    start=True, stop=True)
            et = etp.tile([P, 2, PH], BF16, tag="et", name=f"et_{ph}_{t}")
            nc.scalar.activation(out=et, in_=ps, func=EXP, scale=1.0)
            return et

        def pv_half(po, t, h, et):
            j = 2 * t + h
            for cc in range(C // P):
                nc.tensor.matmul(
                    po[cc],
                    vt_sb[:, j, bass.ts(cc, P)],
                    et[:, h],
                    start=(t == 0 and h == 0), stop=(t == NPAIR - 1 and h == 1))

        def finalize(ph, za, zb, po, et15):
            i0 = ph * PH
            # Z colsum + partition-broadcast via all-ones bf16 lhsT matmuls;
            # the last pair's exp tile is summed directly by the PE (avoids
            # waiting on the accumulation chains).  The reciprocal/scale/add/
            # DMA chain runs on Vector/Pool while the PE begins the next phase
            # (po PSUM banks rotate between phases, so the next phase's PV
            # does not wait on this chain).
            pzb = p_s.tile([P, PH], F32, tag="s", name=f"pzb_{ph}")
            nc.tensor.matmul(pzb, ones_b, za, start=True, stop=False)
            nc.tensor.matmul(pzb, ones_b, zb, start=False, stop=False)
            nc.tensor.matmul(pzb, ones_b, et15[:, 0], start=False, stop=False)
            nc.tensor.matmul(pzb, ones_b, et15[:, 1], start=False, stop=True)
            zbc = zp.tile([P, PH], F32, tag="zbc", name=f"zbc_{ph}")
            scr = zp.tile([P, PH], F32, tag="scr", name=f"scr_{ph}")
            nc.vector.reciprocal_approx_accurate(out=zbc, in_=pzb, scratch=scr)
            sl_i = bass.ds(i0, PH)
            ob = outp.tile([P, 2, PH], F32, tag="ob", name=f"ob_{ph}")
            for cc in range(C // P):
                nc.vector.tensor_mul(ob[:, cc], po[cc], zbc)
                nc.gpsimd.tensor_add(ob[:, cc], ob[:, cc], _f(x_sb[:, cc, sl_i]))
            nc.sync.dma_start(
                out=out_d[:].rearrange("(o p) n -> p o n", p=P)[:, :, sl_i],
                in_=ob)

        pend = {(0, 0): issue_pair(0, 0)}
        for ph in range(NPH):
            za = zp.tile([P, PH], BF16, tag="za", name=f"za_{ph}")
            zb = zp.tile([P, PH], BF16, tag="zb", name=f"zb_{ph}")
            po = [pp.tile([P, PH], F32, tag="pp", name=f"po_{ph}_{cc}")
                  for cc in range(C // P)]
            for t in range(NPAIR):
                et = pend.pop((ph, t))
                pv_half(po, t, 0, et)
                nxt = (ph, t + 1) if t + 1 < NPAIR else (
                    (ph + 1, 0) if ph + 1 < NPH else None)
                if nxt is not None:
                    pend[nxt] = issue_pair(*nxt)
                pv_half(po, t, 1, et)
                if t == NPAIR - 1:
                    finalize(ph, za, zb, po, et)
                elif t == 0:
                    nc.vector.tensor_copy(za, et[:, 0])
                    nc.vector.tensor_copy(zb, et[:, 1])
                else:
                    nc.vector.tensor_add(za, za, et[:, 0])
                    nc.vector.tensor_add(zb, zb, et[:, 1])


def _build_nc():
    nc = bacc.Bacc(trn_type="TRN2", target_bir_lowering=False, debug=False)
    aps = (
        nc.declare_dram_parameter("x", [C, N], MM_DT, isOutput=False),
        nc.declare_dram_parameter("wqT", [C, P], MM_DT, isOutput=False),
        nc.declare_dram_parameter("wkT", [C, P], MM_DT, isOutput=False),
        nc.declare_dram_parameter("wvT", [C, C], MM_DT, isOutput=False),
        nc.declare_dram_parameter("qb", [P, 1], F32, isOutput=False),
        nc.declare_dram_parameter("kb", [P, 1], F32, isOutput=False),
        nc.declare_dram_parameter("vb", [1, C], F32, isOutput=False),
        nc.declare_dram_parameter("gamma", [1, 1], F32, isOutput=False),
        nc.declare_dram_parameter("out", [C, NI], F32, isOutput=True),
    )
    with tile.TileContext(nc) as tc:
        _emit(tc, aps)
    nc.compile()
    return nc


_NC_CACHE = {}


def get_nc():
    if "nc" not in _NC_CACHE:
        _NC_CACHE["nc"] = _build_nc()
    return _NC_CACHE["nc"]


def make_in_maps(inputs):
    """Build the 8 per-core input maps from the full problem inputs."""
    f = np.float32
    x_streams = [
        np.ascontiguousarray(inputs["input1"].reshape(B, C, N), dtype=f),
        np.ascontiguousarray(inputs["input2"].reshape(B, C, N), dtype=f),
    ]
    wsets = []
    for s in ("1", "2"):
        qw = np.asarray(inputs[f"q{s}_w"], dtype=f)
        kw = np.asarray(inputs[f"k{s}_w"], dtype=f)
        vw = np.asarray(inputs[f"v{s}_w"], dtype=f)
        qb = np.asarray(inputs[f"q{s}_b"], dtype=f)
        kb = np.asarray(inputs[f"k{s}_b"], dtype=f)
        vb = np.asarray(inputs[f"v{s}_b"], dtype=f)
        wsets.append(dict(
            wqT=np.ascontiguousarray(np.concatenate([qw, qw], 0).T),
            wkT=np.ascontiguousarray(np.concatenate([kw, kw], 0).T),
            wvT=np.ascontiguousarray(vw.T),
            qb=np.ascontiguousarray(np.concatenate([qb, qb])[:, None]),
            kb=np.ascontiguousarray(np.concatenate([kb, kb])[:, None]),
            vb=np.ascontiguousarray(vb[None, :]),
        ))
    gamma = np.ascontiguousarray(np.asarray(inputs["gamma"], dtype=f).reshape(1, 1))

    in_maps = []
    for core in range(8):
        u, h = core // 2, core % 2
        b, s = u // 2, u % 2
        xs = x_streams[s][b]
        m = dict(wsets[s])
        # rotate so this core's query slice comes first (attention contracts
        # over all keys, so key order is irrelevant)
        if h == 0:
            m["x"] = xs
        else:
            m["x"] = np.ascontiguousarray(
                np.concatenate([xs[:, NI:], xs[:, :NI]], axis=1))
        m["gamma"] = gamma
        in_maps.append(m)
    return in_maps


def assemble(results, inputs):
    """Stitch the 8 per-core [256, 2048] outputs into (out1, out2)."""
    outs = [np.empty((B, C, N), np.float32) for _ in range(2)]
    for core in range(8):
        u, h = core // 2, core % 2
        b, s = u // 2, u % 2
        outs[s][b][:, h * NI:(h + 1) * NI] = results[core]["out"]
    out1 = outs[0].reshape(B, C, H, W)
    out2 = outs[1].reshape(B, C, H, W)
    return out1, out2


def kernel(**inputs):
    from concourse.bass_utils import run_bass_kernel_spmd

    nc = get_nc()
    in_maps = make_in_maps(inputs)
    res = run_bass_kernel_spmd(nc, in_maps, list(range(8)))
    return assemble(res.results, inputs)


# revision 14
# speedup vs baseline: 1.1306x; 1.0374x over previous
"""Trainium2 Bass kernel for nn_AttShare: dual-stream 1x1-conv attention.

Full-input contract: kernel(**inputs) takes the complete tensors from
setup_inputs() and returns (out1, out2) exactly like the reference.

Sharding (8 cores): 4 independent (batch, stream) attention units x 2-way
query-row split.  Each core gets the full x=[256,4096] of its unit, HOST-
ROTATED so its 2048 query columns come first; it produces
out = gamma * (V @ softmax(Q K^T)^T)[:, 0:2048] + x[:, 0:2048].
(Attention contracts over all keys, so the key/value column order is
irrelevant; the host scatters the output back to the right columns.)

Key simplification: the reference adds a per-row bias (q . g) to the logits
before a row-softmax.  softmax is shift-invariant per row, so the entire
global-gating branch (pooled means -> MLP -> sigmoid -> bias) cancels and is
not computed.  The k-projection bias also shifts logits uniformly per row
and cancels; the q bias does not and is applied.  The v bias adds
gamma*vb[c] (softmax rows sum to 1); it is folded into the V^T tiles.

Precision: projections and QK logits run in float32r (full fp32 weights,
~19-bit moving operand) -- logit errors get exponentiated, so this path
stays wide.  The PV path (V^T tiles and exp tiles) runs in bfloat16:
weight loads take 1 pass instead of fp32's 2, cutting the ldweights
exposure between back-to-back PV matmuls.  Measured accuracy impact
~1.4e-3 relative (tolerance 2e-2).

On-core dataflow (per core):
  proj:  qq = Wq_dup @ x[:, :2048] (+qb)  [128, 2048] f32r  (q/k duplicated
         kk = Wk_dup @ x  (+kb)           [128, 4096] f32r   on both halves
         vt = gamma*(x^T @ Wv^T + vb)     [128 j, 32, 256] bf16  for packing)
  attn (2 phases of 1024 query columns, j streamed in row-packed pairs,
        software-pipelined one pair ahead):
         S^T tile = kk_j^T @ qq  (K=64, rows 0-63 / 64-127 concurrently)
         E = exp(S^T)  (ScalarE, PSUM -> bf16 SBUF; no max-shift needed:
                        |S|<~60 and the denominator normalizes later)
         ZA += E_A (Pool)   ZB += E_B (Vector)   [split across engines]
         out_psum[c,i] += vt_j^T @ E  (bf16 matmuls, PSUM-resident)
  finalize per 512-col slice (pipelined against the next phase / the last
  PV matmuls): Z colsum+broadcast via all-ones lhsT matmuls (4 terms: ZA,
  ZB and the last pair's exp tiles summed directly by the PE), reciprocal
  (Vector), out = out_psum * recip (Vector) + x (Pool), DMA out (SP/ACT
  queues alternating).
  PSUM budget 8 banks: 4 output accumulators + 2x2-bank S^T tiles.

Head: input DMA issues are split across the SP and Activation hardware
queues (plus Pool's software queue for the scalars) so descriptor
generation (~0.7us each) does not serialize the x stream.
"""

import os
import sys

import numpy as np

for _p in ("/opt/trn_rl_repo", os.path.expanduser("~/.axon_site/_ro/trn_rl_repo")):
    if os.path.isdir(_p) and _p not in sys.path:
        sys.path.insert(0, _p)

import concourse.bass as bass  # noqa: E402
import concourse.bacc as bacc  # noqa: E402
import concourse.mybir as mybir  # noqa: E402
import concourse.tile as tile  # noqa: E402

P = 128
C = 256         # channels
CQ = 64         # q/k channels
N = 4096        # H*W
NI = 2048       # query rows per core
PH = 512        # query columns processed per phase
B, H, W = 2, 64, 64
F32 = mybir.dt.float32
BF16 = mybir.dt.bfloat16
MM_DT = mybir.dt.float32r


def _f(ap):
    """View a float32r AP as plain fp32 (for non-matmul engine access)."""
    return ap.bitcast(F32)


def _r(ap):
    """View an fp32 AP as float32r (for matmul operands)."""
    return ap.bitcast(MM_DT)


def _emit(tc, aps):
    nc = tc.nc
    import contextlib

    x_d, wq_d, wk_d, wv_d, qb_d, kb_d, vb_d, gamma_d, out_d = aps
    EXP = mybir.ActivationFunctionType.Exp
    IDENT = mybir.ActivationFunctionType.Identity

    with contextlib.ExitStack() as ctx:
        singles = ctx.enter_context(tc.tile_pool(name="singles", bufs=1))
        pp = ctx.enter_context(tc.tile_pool(name="pp", bufs=4, space="PSUM"))
        p_s = ctx.enter_context(tc.tile_pool(name="p_s", bufs=2, space="PSUM"))
        etp = ctx.enter_context(tc.tile_pool(name="etp", bufs=8))
        zp = ctx.enter_context(tc.tile_pool(name="zp", bufs=4))
        outp = ctx.enter_context(tc.tile_pool(name="outp", bufs=4))

        # ---- loads --------------------------------------------------------------
        x_sb = singles.tile([P, 2, N], MM_DT)
        wq_sb = singles.tile([P, 2, P], MM_DT)
        wk_sb = singles.tile([P, 2, P], MM_DT)
        wv_sb = singles.tile([P, 2, C], MM_DT)
        x_r = x_d[:].rearrange("(o p) n -> p o n", p=P)

        gamma_sb = singles.tile([1, 1], F32)
        kb_sb = singles.tile([P, 1], F32)
        qb_sb = singles.tile([P, 1], F32)
        vb_sb = singles.tile([1, C], F32)

        # gamma/vb lead the SP queue (tiny transfers, unblock the HAM-warming
        # broadcast matmuls), then the x stream; qb/kb ride the Pool software
        # queue.  ScalarE issues no DMAs -- it is exp-bound.
        nc.sync.dma_start(out=gamma_sb, in_=gamma_d[:])
        nc.sync.dma_start(out=vb_sb, in_=vb_d[:])
        nc.gpsimd.dma_start(out=qb_sb, in_=qb_d[:])
        nc.gpsimd.dma_start(out=kb_sb, in_=kb_d[:])

        def ld_x(queue, c):
            queue.dma_start(out=x_sb[:, :, bass.ts(c, N // 8)],
                            in_=x_r[:, :, bass.ts(c, N // 8)])

        # first chunk split by cin-half so the first projection operand lands
        # in ~half the transfer time
        nc.sync.dma_start(out=x_sb[:, 0:1, bass.ts(0, N // 8)],
                          in_=x_r[:, 0:1, bass.ts(0, N // 8)])
        nc.sync.dma_start(out=wq_sb, in_=wq_d[:].rearrange("(o p) m -> p o m", p=P))
        nc.sync.dma_start(out=x_sb[:, 1:2, bass.ts(0, N // 8)],
                          in_=x_r[:, 1:2, bass.ts(0, N // 8)])
        nc.sync.dma_start(out=wk_sb, in_=wk_d[:].rearrange("(o p) m -> p o m", p=P))
        nc.sync.dma_start(out=wv_sb, in_=wv_d[:].rearrange("(o p) m -> p o m", p=P))
        # alternate the two hardware DMA rings so the x stream is not limited
        # by a single ring's bandwidth
        for c in range(1, 8):
            ld_x(nc.sync if c % 2 == 0 else nc.scalar, c)

        ones_1 = singles.tile([1, P], F32)    # lhsT for K=1 partition broadcast
        nc.vector.memset(ones_1, 1.0)
        ones_b = singles.tile([P, P], BF16)   # all-ones bf16 lhsT: Z colsum
        nc.vector.memset(ones_b, 1.0)

        # broadcast gamma and gamma*vb across partitions via K=1 matmuls
        gamma_bc = singles.tile([P, 1], F32)
        pg = pp.tile([P, 1], F32, tag="pp", name="pg")
        nc.tensor.matmul(pg, ones_1, gamma_sb, start=True, stop=True)
        nc.vector.tensor_copy(gamma_bc, pg)
        gvb_bc = singles.tile([P, C], F32)
        pvb = pp.tile([P, C], F32, tag="pp")
        nc.tensor.matmul(pvb, ones_1, vb_sb, start=True, stop=True)
        nc.vector.tensor_scalar_mul(gvb_bc, pvb, gamma_bc)

        # ---- projections --------------------------------------------------------
        qq_sb = singles.tile([P, NI], MM_DT)   # [q; q] duplicated across halves
        kk_sb = singles.tile([P, N], MM_DT)    # [k; k] duplicated across halves
        vt_sb = singles.tile([P, N // P, C], BF16)   # V^T: [j, c], pre-scaled

        def qq_slice(s):
            ps = pp.tile([P, 512], F32, tag="pp", name=f"qq_ps_{s}")
            nc.tensor.matmul(ps, wq_sb[:, 0], x_sb[:, 0, bass.ts(s, 512)],
                             start=True, stop=False)
            nc.tensor.matmul(ps, wq_sb[:, 1], x_sb[:, 1, bass.ts(s, 512)],
                             start=False, stop=True)
            nc.scalar.activation(out=qq_sb[:, bass.ts(s, 512)], in_=ps,
                                 func=IDENT, bias=qb_sb, scale=1.0)

        def kk_slice(s):
            ps = pp.tile([P, 512], F32, tag="pp", name=f"kk_ps_{s}")
            nc.tensor.matmul(ps, wk_sb[:, 0], x_sb[:, 0, bass.ts(s, 512)],
                             start=True, stop=False)
            nc.tensor.matmul(ps, wk_sb[:, 1], x_sb[:, 1, bass.ts(s, 512)],
                             start=False, stop=True)
            nc.scalar.activation(out=kk_sb[:, bass.ts(s, 512)], in_=ps,
                                 func=IDENT, bias=kb_sb, scale=1.0)

        def vt_chunk(j):
            ps = pp.tile([P, C], F32, tag="pp", name=f"vt_ps_{j}")
            nc.tensor.matmul(ps, x_sb[:, 0, bass.ts(j, P)], wv_sb[:, 0],
                             start=True, stop=False)
            nc.tensor.matmul(ps, x_sb[:, 1, bass.ts(j, P)], wv_sb[:, 1],
                             start=False, stop=True)
            nc.vector.scalar_tensor_tensor(
                out=vt_sb[:, j], in0=ps, scalar=gamma_bc, in1=gvb_bc,
                op0=mybir.AluOpType.mult, op1=mybir.AluOpType.add)

        # queries are columns 0..NI-1 of the rotated x; consume x strictly in
        # chunk-arrival order (kk slice s and vt chunks 4s..4s+3 share chunk s)
        qq_slice(0)
        qq_slice(1)
        for s in range(N // 512):
            kk_slice(s)
            for j in range(4 * s, 4 * s + 4):
                vt_chunk(j)
            if s == 1:
                qq_slice(2)
            elif s == 2:
                qq_slice(3)

        # ---- attention ----------------------------------------------------------
        # Row-packed QK: pair (jA, jB) = (2t, 2t+1); jA on PE rows 0-63, jB on
        # rows 64-127 (via the duplicated q/k halves), running concurrently.
        NPAIR = N // P // 2   # 16 pairs per phase
        NPH = NI // PH        # 2 phases

        def issue_pair(ph, t):
            # One PSUM tile holds both halves' S^T slices ([P, 2, 512]); the
            # two K=64 QK matmuls row-pack (rows 0-63 / 64-127) and a SINGLE
            # [128, 1024] exp covers both halves (amortizes ScalarE's fixed
            # per-instruction overhead -- ScalarE is the near-critical engine).
            i0 = ph * PH
            ps = p_s.tile([P, 2, PH], F32, tag="s", name=f"ps_{ph}_{t}")
            for h, j in ((0, 2 * t), (1, 2 * t + 1)):
                lo = h * CQ
                nc.tensor.matmul(
                    ps[:, h],
                    kk_sb[lo:lo + CQ, bass.ts(j, P)],
                    qq_sb[lo:lo + CQ, bass.ds(i0, PH)],
                    start=True, stop=True)
            et = etp.tile([P, 2, PH], BF16, tag="et", name=f"et_{ph}_{t}")
            nc.scalar.activation(out=et, in_=ps, func=EXP, scale=1.0)
            return et

        def pv_half(po, t, h, et):
            j = 2 * t + h
            for cc in range(C // P):
                nc.tensor.matmul(
                    po[cc],
                    vt_sb[:, j, bass.ts(cc, P)],
                    et[:, h],
                    start=(t == 0 and h == 0), stop=(t == NPAIR - 1 and h == 1))

        def finalize(ph, za, zb, po, et15):
            i0 = ph * PH
            # Z colsum + partition-broadcast via all-ones bf16 lhsT matmuls;
            # the last pair's exp tile is summed directly by the PE (avoids
            # waiting on the accumulation chains).  The reciprocal/scale/add/
            # DMA chain runs on Vector/Pool while the PE begins the next phase
            # (po PSUM banks rotate between phases, so the next phase's PV
            # does not wait on this chain).
            pzb = p_s.tile([P, PH], F32, tag="s", name=f"pzb_{ph}")
            nc.tensor.matmul(pzb, ones_b, za, start=True, stop=False)
            nc.tensor.matmul(pzb, ones_b, zb, start=False, stop=False)
            nc.tensor.matmul(pzb, ones_b, et15[:, 0], start=False, stop=False)
            nc.tensor.matmul(pzb, ones_b, et15[:, 1], start=False, stop=True)
            zbc = zp.tile([P, PH], F32, tag="zbc", name=f"zbc_{ph}")
            scr = zp.tile([P, PH], F32, tag="scr", name=f"scr_{ph}")
            nc.vector.reciprocal_approx_accurate(out=zbc, in_=pzb, scratch=scr)
            sl_i = bass.ds(i0, PH)
            ob = outp.tile([P, 2, PH], F32, tag="ob", name=f"ob_{ph}")
            for cc in range(C // P):
                nc.vector.tensor_mul(ob[:, cc], po[cc], zbc)
                nc.gpsimd.tensor_add(ob[:, cc], ob[:, cc], _f(x_sb[:, cc, sl_i]))
            nc.sync.dma_start(
                out=out_d[:].rearrange("(o p) n -> p o n", p=P)[:, :, sl_i],
                in_=ob)

        pend = {(0, 0): issue_pair(0, 0)}
        for ph in range(NPH):
            za = zp.tile([P, PH], BF16, tag="za", name=f"za_{ph}")
            zb = zp.tile([P, PH], BF16, tag="zb", name=f"zb_{ph}")
            po = [pp.tile([P, PH], F32, tag="pp", name=f"po_{ph}_{cc}")
                  for cc in range(C // P)]
            for t in range(NPAIR):
                et = pend.pop((ph, t))
                nxt = (ph, t + 1) if t + 1 < NPAIR else (
                    (ph + 1, 0) if ph + 1 < NPH else None)
                if nxt is not None:
                    pend[nxt] = issue_pair(*nxt)
                pv_half(po, t, 0, et)
                pv_half(po, t, 1, et)
                if t == NPAIR - 1:
                    finalize(ph, za, zb, po, et)
                elif t == 0:
                    nc.vector.tensor_copy(za, et[:, 0])
                    nc.vector.tensor_copy(zb, et[:, 1])
                else:
                    nc.vector.tensor_add(za, za, et[:, 0])
                    nc.vector.tensor_add(zb, zb, et[:, 1])


def _build_nc():
    nc = bacc.Bacc(trn_type="TRN2", target_bir_lowering=False, debug=False)
    aps = (
        nc.declare_dram_parameter("x", [C, N], MM_DT, isOutput=False),
        nc.declare_dram_parameter("wqT", [C, P], MM_DT, isOutput=False),
        nc.declare_dram_parameter("wkT", [C, P], MM_DT, isOutput=False),
        nc.declare_dram_parameter("wvT", [C, C], MM_DT, isOutput=False),
        nc.declare_dram_parameter("qb", [P, 1], F32, isOutput=False),
        nc.declare_dram_parameter("kb", [P, 1], F32, isOutput=False),
        nc.declare_dram_parameter("vb", [1, C], F32, isOutput=False),
        nc.declare_dram_parameter("gamma", [1, 1], F32, isOutput=False),
        nc.declare_dram_parameter("out", [C, NI], F32, isOutput=True),
    )
    with tile.TileContext(nc) as tc:
        _emit(tc, aps)
    nc.compile()
    return nc


_NC_CACHE = {}


def get_nc():
    if "nc" not in _NC_CACHE:
        _NC_CACHE["nc"] = _build_nc()
    return _NC_CACHE["nc"]


def make_in_maps(inputs):
    """Build the 8 per-core input maps from the full problem inputs."""
    f = np.float32
    x_streams = [
        np.ascontiguousarray(inputs["input1"].reshape(B, C, N), dtype=f),
        np.ascontiguousarray(inputs["input2"].reshape(B, C, N), dtype=f),
    ]
    wsets = []
    for s in ("1", "2"):
        qw = np.asarray(inputs[f"q{s}_w"], dtype=f)
        kw = np.asarray(inputs[f"k{s}_w"], dtype=f)
        vw = np.asarray(inputs[f"v{s}_w"], dtype=f)
        qb = np.asarray(inputs[f"q{s}_b"], dtype=f)
        kb = np.asarray(inputs[f"k{s}_b"], dtype=f)
        vb = np.asarray(inputs[f"v{s}_b"], dtype=f)
        wsets.append(dict(
            wqT=np.ascontiguousarray(np.concatenate([qw, qw], 0).T),
            wkT=np.ascontiguousarray(np.concatenate([kw, kw], 0).T),
            wvT=np.ascontiguousarray(vw.T),
            qb=np.ascontiguousarray(np.concatenate([qb, qb])[:, None]),
            kb=np.ascontiguousarray(np.concatenate([kb, kb])[:, None]),
            vb=np.ascontiguousarray(vb[None, :]),
        ))
    gamma = np.ascontiguousarray(np.asarray(inputs["gamma"], dtype=f).reshape(1, 1))

    in_maps = []
    for core in range(8):
        u, h = core // 2, core % 2
        b, s = u // 2, u % 2
        xs = x_streams[s][b]
        m = dict(wsets[s])
        # rotate so this core's query slice comes first (attention contracts
        # over all keys, so key order is irrelevant)
        if h == 0:
            m["x"] = xs
        else:
            m["x"] = np.ascontiguousarray(
                np.concatenate([xs[:, NI:], xs[:, :NI]], axis=1))
        m["gamma"] = gamma
        in_maps.append(m)
    return in_maps


def assemble(results, inputs):
    """Stitch the 8 per-core [256, 2048] outputs into (out1, out2)."""
    outs = [np.empty((B, C, N), np.float32) for _ in range(2)]
    for core in range(8):
        u, h = core // 2, core % 2
        b, s = u // 2, u % 2
        outs[s][b][:, h * NI:(h + 1) * NI] = results[core]["out"]
    out1 = outs[0].reshape(B, C, H, W)
    out2 = outs[1].reshape(B, C, H, W)
    return out1, out2


def kernel(**inputs):
    from concourse.bass_utils import run_bass_kernel_spmd

    nc = get_nc()
    in_maps = make_in_maps(inputs)
    res = run_bass_kernel_spmd(nc, in_maps, list(range(8)))
    return assemble(res.results, inputs)
